# revision 68
# baseline (speedup 1.0000x reference)
"""LIF multicompartment refractory cell step on 8 Trainium2 NeuronCores.

Data-parallel over batch: each core handles B_LOC=512 of B=4096 rows.
On-device layout is transposed ([H, B_loc]) and fully host-preswizzled so
every DMA is a flat [128, X] transfer. The hidden/contraction dim sits on
SBUF partitions, so the three GEMMs need no on-device transposes.

Two-phase schedule, ordered so the startup is fed by the cheapest bytes
(bf16/fp8 weights carry 2-4x the PE-work per DMA byte of f32):

  phase A:  ps2x = inp @ Wi.T      (bf16; partial sums parked in SBUF
                                    as bf16 via ACT-engine evacuation)
  phase B:  ps2z = z @ Wr.T        (fp8 e4m3 + DoubleRow perf mode:
                                    host-scaled z*16 / Wr*64, 2 k-tiles
                                    per MM at half the cycles, rescaled
                                    by 1/1024 at evacuation)
            ps1  = v @ g_coupling.T (f32r — feeds the spike threshold,
                                    needs ~fp32 accuracy)
            + the LIF/refractory elementwise tail per h-pair on DVE/ACT,
            outputs streamed via the ACT HWDGE queue (SP queue stays
            load-only so weight prefetch never head-of-line blocks).

Dtype budget per core (46 MiB HBM vs 84 all-f32): v,w1 stay f32
(threshold path), i,x,w2x,io,ro,vo bf16, z,w2z fp8, zo uint8; rho ships
as a host-precomputed uint8 refrac mask + uint8-quantized relu(rho-1)
base (rho appears nowhere else on device); the inverted mask and the
dequantized base are derived on the idle ACT engine (Copy with
scale/bias) so the DVE chain stays under the DMA pace. Hardware-measured
total 1.216e-2 vs the 2e-2 gate (f32r GEMM noise + bf16-i flips, 177
z flips + fp8 i_new noise; fp8 on the x-half or v, or bf16 v, would
blow the budget — measured, rejected).

Elementwise identities: rho_new = relu(rho-1) then 5 where z_new=1
(copy_predicated, uint8 masks — the BIR verifier requires int masks);
v_new = (vdec<=1)*vdec then held v where refractory.

Sim (CoreSim cost model): 142 us/core vs 281 baseline; DMA-bound at
46 MiB / 360 GB/s with PE 83% busy. Note the cost model rates DoubleRow
at 4x bf16 while hardware measures ~1.4-1.8x, so real HW sits near the
PE/DMA crossover (~140-150 us).
"""
import os
import numpy as np

import concourse.bacc as bacc
import concourse.mybir as mybir
import concourse.tile as tile
from concourse import bass_utils

B, I, H = 4096, 2048, 2048
NCORES = 8
B_LOC = B // NCORES          # 512
HT = H // 128                # 16 h-tiles
HP = HT // 2                 # 8 h-pairs
KT1 = H // 128               # 16 k-tiles, coupling GEMM
KT2 = (I + H) // 128         # 32 k-tiles, i_new GEMM

OUT_NAMES = ["zo", "vo", "io", "ro"]

F32 = np.float32
BF16 = mybir.dt.np(mybir.dt.bfloat16)
FP8 = mybir.dt.np(mybir.dt.float8e4)
Z_SCALE = 16.0
W_SCALE = 64.0
BT_MAX = 4.5
BT_QSCALE = 255.0 / BT_MAX

_cache = {}


def build(num_devices=NCORES):
    nc = bacc.Bacc("TRN2", target_bir_lowering=False, debug=False,
                   num_devices=num_devices)
    f32 = mybir.dt.float32
    f32r = mybir.dt.float32r
    bf16 = mybir.dt.bfloat16
    A = mybir.AluOpType
    BT_SCALE = BT_MAX / 255.0
    F = mybir.ActivationFunctionType

    # activations, host layout [p, kt, b] flattened -> [128, KT1*B_LOC]
    fp8 = mybir.dt.float8e4
    vt_d = nc.dram_tensor("vt", [128, KT1 * B_LOC], f32r, kind="ExternalInput")
    zt_d = nc.dram_tensor("zt", [128, KT1 * B_LOC], fp8, kind="ExternalInput")
    xt_d = nc.dram_tensor("xt", [128, KT1 * B_LOC], bf16, kind="ExternalInput")
    # per-pair state streams, same [p, ht, b] swizzle
    it_d = nc.dram_tensor("it", [128, HT * B_LOC], bf16, kind="ExternalInput")
    mt_d = nc.dram_tensor("mt", [128, HT * B_LOC], mybir.dt.uint8,
                          kind="ExternalInput")
    bt_d = nc.dram_tensor("bt", [128, HT * B_LOC], mybir.dt.uint8,
                          kind="ExternalInput")
    # weights pre-swizzled: [p, ht, kt, c] -> [128, HT*KT*128]
    w1_d = nc.dram_tensor("w1", [128, HT * KT1 * 128], f32r, kind="ExternalInput")
    w2x_d = nc.dram_tensor("w2x", [128, HT * KT1 * 128], bf16,
                           kind="ExternalInput")
    w2z_d = nc.dram_tensor("w2z", [128, HT * KT1 * 128], fp8,
                           kind="ExternalInput")

    zo_d = nc.dram_tensor("zo", [128, HT * B_LOC], mybir.dt.uint8,
                          kind="ExternalOutput")
    vo_d = nc.dram_tensor("vo", [128, HT * B_LOC], bf16, kind="ExternalOutput")
    io_d = nc.dram_tensor("io", [128, HT * B_LOC], bf16, kind="ExternalOutput")
    ro_d = nc.dram_tensor("ro", [128, HT * B_LOC], bf16, kind="ExternalOutput")

    with tile.TileContext(nc) as tc:
        with (
            tc.tile_pool(name="resid", bufs=1) as resid,
            tc.tile_pool(name="w1pool", bufs=5) as w1pool,
            tc.tile_pool(name="w2xpool", bufs=5) as w2xpool,
            tc.tile_pool(name="w2zpool", bufs=4) as w2zpool,
            tc.tile_pool(name="spool", bufs=3) as spool,
            tc.tile_pool(name="epool", bufs=2) as epool,
            tc.tile_pool(name="opool", bufs=2) as opool,
            tc.tile_pool(name="pspool", bufs=2, space="PSUM") as pspool,
        ):
            u8 = mybir.dt.uint8
            vt_sb = resid.tile([128, KT1 * B_LOC], f32r)
            zt_sb = resid.tile([128, KT1, B_LOC], fp8)
            xt_sb = resid.tile([128, KT1 * B_LOC], bf16)
            p2x_sb = resid.tile([128, HT * B_LOC], bf16)
            fives = resid.tile([128, 2 * B_LOC], bf16)
            nc.vector.memset(fives[:], 5.0)
            zeros = resid.tile([128, 2 * B_LOC], u8)
            nc.vector.memset(zeros[:], 0)

            # ---- helpers ----
            KB = B_LOC                # one k-tile worth of rhs columns
            w1_t = {}
            w2x_t = {}
            w2z_t = {}
            i2_t = {}
            r2_t = {}
            ps1_t = {}
            ps2_t = {}

            def load_w1(h):
                t = w1pool.tile([128, KT1 * 128], f32r, name="w1_sb")
                nc.sync.dma_start(
                    t[:], w1_d[:, h * KT1 * 128:(h + 1) * KT1 * 128])
                w1_t[h] = t

            def load_w2(h, half):
                off = h * KT1 * 128
                if half == 0:
                    t = w2xpool.tile([128, KT1 * 128], bf16, name="w2_sb")
                    nc.sync.dma_start(
                        t[:], w2x_d[:, off:off + KT1 * 128])
                    w2x_t[h] = t
                else:
                    t = w2zpool.tile([128, KT1, 128], fp8, name="w2z_sb")
                    nc.sync.dma_start(
                        t[:, :, :], w2z_d[:, off:off + KT1 * 128])
                    w2z_t[h] = t

            def load_states(hp):
                pw = slice(2 * hp * B_LOC, (2 * hp + 2) * B_LOC)
                i2 = spool.tile([128, 2 * B_LOC], bf16, name="i2")
                nc.sync.dma_start(i2[:], it_d[:, pw])
                i2_t[hp] = i2
                m2i = spool.tile([128, 2 * B_LOC], u8, name="m2i")
                nc.sync.dma_start(m2i[:], mt_d[:, pw])
                b2u = spool.tile([128, 2 * B_LOC], u8, name="b2u")
                nc.sync.dma_start(b2u[:], bt_d[:, pw])
                r2_t[hp] = (m2i, b2u)

            def derive_states(hp):
                """Invert the refrac mask and dequantize the rho base on
                DVE (cheap ops traded for 2 MiB of HBM traffic). Called
                at pair-body start, well after the DMAs landed, so the
                in-order DVE queue never stalls on them."""
                m2i, b2u = r2_t[hp]
                n2i = spool.tile([128, 2 * B_LOC], u8, name="n2i")
                nc.scalar.activation(n2i[:], m2i[:], F.Copy,
                                     bias=1.0, scale=-1.0)
                rob = spool.tile([128, 2 * B_LOC], bf16, name="rob")
                nc.scalar.activation(rob[:], b2u[:], F.Copy,
                                     bias=0.0, scale=BT_SCALE)
                r2_t[hp] = (m2i, n2i, rob)

            def mm1(h):
                """ps1[h] = v @ g_coupling.T accumulation, all k-tiles."""
                ps1 = pspool.tile([128, B_LOC], f32, name="ps1", bufs=4)
                ps1_t[h] = ps1
                w1_sb = w1_t.pop(h)
                for k in range(KT1):
                    nc.tensor.matmul(
                        ps1[:], w1_sb[:, k * 128:(k + 1) * 128],
                        vt_sb[:, k * KB:(k + 1) * KB],
                        start=(k == 0), stop=(k == KT1 - 1))

            def mm2(h, half, k0, k1):
                """x-part (half=0) or z-part (half=1) of the i_new GEMM.
                The z-part runs in fp8 e4m3 with DoubleRow (2 weights per
                PE cell, two k-tiles per instruction at 0.5 cyc/row)."""
                name = "ps2x" if half == 0 else "ps2z"
                if (h, half) not in ps2_t:
                    ps2_t[(h, half)] = pspool.tile(
                        [128, B_LOC], f32, name=name)
                pso = ps2_t[(h, half)]
                if half == 0:
                    w2_sb = w2x_t[h]
                    for k in range(k0, k1):
                        nc.tensor.matmul(
                            pso[:], w2_sb[:, k * 128:(k + 1) * 128],
                            xt_sb[:, k * KB:(k + 1) * KB],
                            start=(k == 0), stop=(k == KT1 - 1))
                else:
                    w2_sb = w2z_t[h]
                    for j in range(k0 // 2, k1 // 2):
                        nc.tensor.matmul(
                            pso[:], w2_sb[:, 2 * j:2 * j + 2, :],
                            zt_sb[:, 2 * j:2 * j + 2, :],
                            start=(j == 0), stop=(j == KT1 // 2 - 1),
                            perf_mode=mybir.MatmulPerfMode.DoubleRow)

            def evac_x(h):
                """Park the x-part partial sum in SBUF (bf16), on ACT."""
                hw = slice(h * B_LOC, (h + 1) * B_LOC)
                nc.scalar.activation(p2x_sb[:, hw], ps2_t.pop((h, 0))[:],
                                     F.Copy, bias=0.0, scale=1.0)
                w2x_t.pop(h)

            # ---- phase A: x-half of the i_new GEMM (bf16, DMA-dense;
            # partial sums parked in SBUF so only 2 PSUM banks are used) ----
            load_w2(0, 0)
            for c in range(2):
                nc.sync.dma_start(xt_sb[:, 2 * c * KB:2 * (c + 1) * KB],
                                  xt_d[:, 2 * c * KB:2 * (c + 1) * KB])
                mm2(0, 0, 2 * c, 2 * (c + 1))
            load_w2(1, 0)
            for c in range(2, 8):
                nc.sync.dma_start(xt_sb[:, 2 * c * KB:2 * (c + 1) * KB],
                                  xt_d[:, 2 * c * KB:2 * (c + 1) * KB])
                mm2(0, 0, 2 * c, 2 * (c + 1))
            mm2(1, 0, 0, KT1)
            evac_x(0)
            for h in range(2, HT):
                load_w2(h, 0)
                mm2(h, 0, 0, KT1)
                evac_x(h - 1)
            evac_x(HT - 1)

            # ---- phase B: z-half + coupling GEMMs + LIF elementwise ----
            load_w1(0)
            load_w2(0, 1)
            nc.sync.dma_start(zt_sb[:, :8, :], zt_d[:, :8 * KB])
            nc.sync.dma_start(zt_sb[:, 8:, :], zt_d[:, 8 * KB:])
            load_w2(1, 1)
            load_states(0)
            for c in range(4):
                nc.sync.dma_start(vt_sb[:, 4 * c * KB:4 * (c + 1) * KB],
                                  vt_d[:, 4 * c * KB:4 * (c + 1) * KB])
            load_w1(1)
            load_states(1)

            u2_t = {}

            def act_u2(hp):
                """u2 = 0.1*i on ACT, prefetched a pair ahead."""
                u2 = epool.tile([128, 2 * B_LOC], f32, name="u2")
                pw = slice(2 * hp * B_LOC, (2 * hp + 2) * B_LOC)
                nc.scalar.activation(u2[:], i2_t[hp][:], F.Copy,
                                     bias=0.0, scale=0.1)
                u2_t[hp] = u2

            act_u2(0)

            def elem_io(hp, hh0, hh1, i2):
                """i_new = 0.8*i + ps2z + parked x-part; frees ps2z banks."""
                n = (hh1 - hh0) * B_LOC
                pw = slice((2 * hp + hh0) * B_LOC,
                           (2 * hp + hh1) * B_LOC)
                iob = opool.tile([128, n], bf16, name="iob")
                for hh in range(hh0, hh1):
                    h = 2 * hp + hh
                    dst = slice((hh - hh0) * B_LOC, (hh - hh0 + 1) * B_LOC)
                    src = slice(hh * B_LOC, (hh + 1) * B_LOC)
                    t = epool.tile([128, B_LOC], f32, name="zt_des")
                    nc.vector.scalar_tensor_tensor(
                        t[:], in0=ps2_t.pop((h, 1))[:], scalar=1.0 / 1024.0,
                        in1=p2x_sb[:, h * B_LOC:(h + 1) * B_LOC],
                        op0=A.mult, op1=A.add)
                    nc.vector.scalar_tensor_tensor(
                        iob[:, dst], in0=i2[:, src], scalar=0.8,
                        in1=t[:], op0=A.mult, op1=A.add)
                nc.scalar.dma_start(io_d[:, pw], iob[:])

            def elem(hp, hh0, hh1, i2, r2, u2, skip_io=False):
                """LIF/refrac elementwise for h-range [2hp+hh0, 2hp+hh1)."""
                n = (hh1 - hh0) * B_LOC
                sw = slice(hh0 * B_LOC, hh1 * B_LOC)            # pair-local
                pw = slice((2 * hp + hh0) * B_LOC,
                           (2 * hp + hh1) * B_LOC)              # global
                v2 = vt_sb[:, pw].bitcast(f32)
                m2i_p, n2i_p, rob_p = r2
                m2i = m2i_p[:, sw]
                n2i = n2i_p[:, sw]
                rob = rob_p[:, sw]

                if not skip_io:
                    elem_io(hp, hh0, hh1, i2)

                # vdec = 0.1*i + ps1   (ps1 already carries 0.9*v via
                # the identity folded into the coupling weights)
                vdec = epool.tile([128, n], f32, name="vdec")
                for hh in range(hh0, hh1):
                    hw = slice((hh - hh0) * B_LOC, (hh - hh0 + 1) * B_LOC)
                    uw = slice(hh * B_LOC, (hh + 1) * B_LOC)
                    nc.vector.tensor_add(vdec[:, hw], u2[:, uw],
                                         ps1_t.pop(2 * hp + hh)[:])

                # spikes: z = (vdec > 1)*not_refrac in one fused stt,
                # using the host-shipped inverted mask
                z2i = epool.tile([128, n], u8, name="z2i")
                nc.vector.scalar_tensor_tensor(
                    z2i[:], in0=vdec[:], scalar=1.0, in1=n2i,
                    op0=A.is_gt, op1=A.mult)

                # v_new = (vdec<=1)*vdec, held v where refractory; the
                # predicated copy needs matching f32 dtypes, so narrow to
                # the bf16 output tile on the (idle) ACT engine afterwards
                vof = epool.tile([128, n], f32, name="vof")
                nc.vector.scalar_tensor_tensor(
                    vof[:], in0=vdec[:], scalar=1.0, in1=vdec[:],
                    op0=A.is_le, op1=A.mult)
                nc.vector.copy_predicated(vof[:], m2i, v2)
                vob = opool.tile([128, n], bf16, name="vob")
                nc.scalar.activation(vob[:], vof[:], F.Copy,
                                     bias=0.0, scale=1.0)

                # rho_new = relu(rho - 1) (host-precomputed), 5 where spiking
                nc.vector.copy_predicated(rob, z2i[:], fives[:, :n])

                if hp == HP - 1:
                    # final outputs over both HWDGE queues for a short tail
                    nc.scalar.dma_start(zo_d[:, pw], z2i[:])
                    nc.sync.dma_start(vo_d[:, pw], vob[:])
                    nc.scalar.dma_start(ro_d[:, pw], rob)
                else:
                    nc.scalar.dma_start(zo_d[:, pw], z2i[:])
                    nc.scalar.dma_start(vo_d[:, pw], vob[:])
                    nc.scalar.dma_start(ro_d[:, pw], rob)

            for hp in range(HP):
                i2 = i2_t.pop(hp)
                derive_states(hp)
                r2 = r2_t.pop(hp)
                u2 = u2_t.pop(hp)
                last = hp == HP - 1

                # prefetch next pair's weights/states
                if hp + 1 < HP:
                    load_w1(2 * hp + 2)
                    load_w2(2 * hp + 2, 1)
                mm2(2 * hp, 1, 0, KT1)
                mm1(2 * hp)
                if hp + 1 < HP:
                    load_w1(2 * hp + 3)
                    load_w2(2 * hp + 3, 1)
                if hp + 2 < HP:
                    load_states(hp + 2)
                if hp + 1 < HP:
                    act_u2(hp + 1)
                if last:
                    # split the final pair per-h so h14's elementwise
                    # overlaps h15's GEMM, and make the h15 z-GEMM the
                    # very last PE work — its only consumer is the cheap
                    # io evacuation, so the kernel tail is short
                    elem(hp, 0, 1, i2, r2, u2)
                    mm1(2 * hp + 1)
                    elem(hp, 1, 2, i2, r2, u2, skip_io=True)
                    mm2(2 * hp + 1, 1, 0, KT1)
                    elem_io(hp, 1, 2, i2)
                else:
                    mm2(2 * hp + 1, 1, 0, KT1)
                    mm1(2 * hp + 1)
                    elem(hp, 0, 2, i2, r2, u2)

    nc.compile()
    return nc


def _sw_act(x, dtype=F32):
    """[B_LOC, K] -> [128, KT*B_LOC] with layout [p, kt, b]."""
    a = np.ascontiguousarray(x.T).reshape(KT1, 128, B_LOC).transpose(1, 0, 2)
    return np.ascontiguousarray(a).astype(dtype).reshape(128, KT1 * B_LOC)


def _unsw(y):
    """[128, HT*B_LOC] ([p, ht, b]) -> [B_LOC, H] float32."""
    a = y.astype(np.float32).reshape(128, HT, B_LOC)
    a = a.transpose(1, 0, 2).reshape(H, B_LOC)
    return a.T


def _sw_w(WT, kt, dtype=F32):
    """WT=[K,H] -> [128, HT*kt*128] with layout [p, ht, kt, c]."""
    a = WT.reshape(kt, 128, HT, 128)              # [k, p, h, c]
    return np.ascontiguousarray(
        a.transpose(1, 2, 0, 3)).astype(dtype).reshape(128, HT * kt * 128)


def swizzle_weights(input_weights, recurrent_weights, g_coupling):
    # fold the 0.9*v membrane-leak term into the coupling GEMM:
    # ps1 = v @ (G + 0.9 I).T = v @ G.T + 0.9 v (saves a DVE op per pair;
    # the f32r rounding on the diagonal adds ~2e-4*|v|, a few flips)
    G = np.ascontiguousarray(
        np.asarray(g_coupling, np.float32).T
        + 0.9 * np.eye(H, dtype=np.float32))
    Wx = np.ascontiguousarray(np.asarray(input_weights, np.float32).T)
    Wz = np.ascontiguousarray(
        np.asarray(recurrent_weights, np.float32).T * W_SCALE)
    return _sw_w(G, KT1), _sw_w(Wx, KT1, BF16), _sw_w(Wz, KT1, FP8)


def make_core_inputs(inp, z, v, i, rho, Wi=None, Wr=None, G=None, w=None):
    """Per-core in_map from this core's [B_LOC, *] shards + full weights."""
    if w is None:
        w = swizzle_weights(Wi, Wr, G)
    w1, w2 = w[0], (w[1], w[2])
    return {
        "vt": _sw_act(v), "zt": _sw_act(z * Z_SCALE, FP8),
        "xt": _sw_act(inp, BF16),
        "it": _sw_act(i, BF16),
        "mt": _sw_act((rho > 0), np.uint8),
        "bt": _sw_act(np.clip(np.round(
            np.maximum(rho - 1.0, 0.0) * BT_QSCALE), 0, 255), np.uint8),
        "w1": w1, "w2x": w2[0], "w2z": w2[1],
    }


def unpack_core_outputs(outs):
    """Dict of raw per-core outputs -> (z_new, v_new, i_new, rho_new)."""
    return tuple(_unsw(outs[name]) for name in OUT_NAMES)


def kernel(inp, z, v, i, rho, input_weights, recurrent_weights, g_coupling):
    inp = np.ascontiguousarray(inp, dtype=np.float32)
    z = np.ascontiguousarray(z, dtype=np.float32)
    v = np.ascontiguousarray(v, dtype=np.float32)
    i = np.ascontiguousarray(i, dtype=np.float32)
    rho = np.ascontiguousarray(rho, dtype=np.float32)

    if "nc" not in _cache:
        _cache["nc"] = build()
    nc = _cache["nc"]
    wkey = (id(input_weights), id(recurrent_weights), id(g_coupling))
    if _cache.get("wkey") != wkey:
        _cache["w"] = swizzle_weights(
            input_weights, recurrent_weights, g_coupling)
        _cache["wkey"] = wkey

    def sw_all(x, dtype=F32):
        """[B, K] -> per-core [128, KT*B_LOC] swizzles in one pass."""
        a = np.ascontiguousarray(x.T).reshape(KT1, 128, NCORES, B_LOC)
        a = np.ascontiguousarray(a.transpose(2, 1, 0, 3)).astype(dtype)
        return a.reshape(NCORES, 128, KT1 * B_LOC)

    sv, sz, sx = sw_all(v), sw_all(z * Z_SCALE, FP8), sw_all(inp, BF16)
    si = sw_all(i, BF16)
    sm = sw_all((rho > 0), np.uint8)
    sb = sw_all(np.clip(np.round(
        np.maximum(rho - 1.0, 0.0) * BT_QSCALE), 0, 255), np.uint8)
    w1, w2x, w2z = _cache["w"]
    in_maps = [{"vt": sv[c], "zt": sz[c], "xt": sx[c],
                "it": si[c], "mt": sm[c], "bt": sb[c],
                "w1": w1, "w2x": w2x, "w2z": w2z}
               for c in range(NCORES)]

    res = bass_utils.run_bass_kernel_spmd(
        nc, in_maps, core_ids=list(range(NCORES)),
        trace=bool(int(os.environ.get("LIF_TRACE", "0"))),
    )
    _cache["last_results"] = res

    outs = []
    for name in OUT_NAMES:
        full = np.empty((B, H), np.float32)
        for c in range(NCORES):
            full[c * B_LOC:(c + 1) * B_LOC] = _unsw(res.results[c][name])
        outs.append(full)
    return np.stack(outs)


# revision 73
# speedup vs baseline: 1.0453x; 1.0453x over previous
"""LIF multicompartment refractory cell step on 8 Trainium2 NeuronCores.

Data-parallel over batch: each core handles B_LOC=512 of B=4096 rows.
On-device layout is transposed ([H, B_loc]) and fully host-preswizzled so
every DMA is a flat [128, X] transfer. The hidden/contraction dim sits on
SBUF partitions, so the three GEMMs need no on-device transposes.

Two-phase schedule, ordered so the startup is fed by the cheapest bytes
(bf16/fp8 weights carry 2-4x the PE-work per DMA byte of f32):

  phase A:  ps2x = inp @ Wi.T      (bf16; partial sums parked in SBUF
                                    as bf16 via ACT-engine evacuation)
  phase B:  ps2z = z @ Wr.T        (fp8 e4m3 + DoubleRow perf mode:
                                    host-scaled z*16 / Wr*64, 2 k-tiles
                                    per MM at half the cycles, rescaled
                                    by 1/1024 at evacuation)
            ps1  = v @ (g_coupling + 0.9 I).T (f32r — feeds the spike
                                    threshold; the folded identity carries
                                    the 0.9*v membrane-leak term)
            + the LIF/refractory elementwise tail per h-pair on DVE/ACT,
            outputs streamed via the ACT HWDGE queue (SP queue stays
            load-only so weight prefetch never head-of-line blocks).

Dtype budget per core (46 MiB HBM vs 84 all-f32): v,w1 stay f32
(threshold path), i,x,w2x,io,ro,vo bf16, z,w2z fp8, zo uint8; rho ships
as a host-precomputed uint8 refrac mask + uint8-quantized relu(rho-1)
base (rho appears nowhere else on device); the inverted mask and the
dequantized base are derived on the idle ACT engine (Copy with
scale/bias) so the DVE chain stays under the DMA pace. Hardware-measured
total 1.241e-2 vs the 2e-2 gate (f32r GEMM noise + bf16-i/diag-fold
flips, 190 z flips + fp8 i_new noise; fp8 on the x-half or v, or bf16
v, would blow the budget — measured, rejected).

Elementwise identities: rho_new = relu(rho-1) then 5 where z_new=1
(copy_predicated, uint8 masks — the BIR verifier requires int masks);
v_new = (vdec<=1)*vdec then held v where refractory.

Sim (CoreSim cost model): 137 us/core vs 281 baseline; SP-load chain
(126 us) and PE (125 us busy, 91%) co-critical at 46 MiB / 360 GB/s. Note the cost model rates DoubleRow
at 4x bf16 while hardware measures ~1.4-1.8x, so real HW sits near the
PE/DMA crossover (~140-150 us).
"""
import os
import numpy as np

import concourse.bacc as bacc
import concourse.mybir as mybir
import concourse.tile as tile
from concourse import bass_utils

B, I, H = 4096, 2048, 2048
NCORES = 8
B_LOC = B // NCORES          # 512
HT = H // 128                # 16 h-tiles
HP = HT // 2                 # 8 h-pairs
KT1 = H // 128               # 16 k-tiles, coupling GEMM
KT2 = (I + H) // 128         # 32 k-tiles, i_new GEMM

OUT_NAMES = ["zo", "vo", "io", "ro"]

F32 = np.float32
BF16 = mybir.dt.np(mybir.dt.bfloat16)
FP8 = mybir.dt.np(mybir.dt.float8e4)
Z_SCALE = 16.0
W_SCALE = 64.0
BT_MAX = 4.5
BT_QSCALE = 255.0 / BT_MAX

_cache = {}


def build(num_devices=NCORES):
    nc = bacc.Bacc("TRN2", target_bir_lowering=False, debug=False,
                   num_devices=num_devices)
    f32 = mybir.dt.float32
    f32r = mybir.dt.float32r
    bf16 = mybir.dt.bfloat16
    A = mybir.AluOpType
    BT_SCALE = BT_MAX / 255.0
    F = mybir.ActivationFunctionType

    # activations, host layout [p, kt, b] flattened -> [128, KT1*B_LOC]
    fp8 = mybir.dt.float8e4
    vt_d = nc.dram_tensor("vt", [128, KT1 * B_LOC], f32r, kind="ExternalInput")
    zt_d = nc.dram_tensor("zt", [128, KT1 * B_LOC], fp8, kind="ExternalInput")
    xt_d = nc.dram_tensor("xt", [128, KT1 * B_LOC], bf16, kind="ExternalInput")
    # per-pair state streams, same [p, ht, b] swizzle
    it_d = nc.dram_tensor("it", [128, HT * B_LOC], bf16, kind="ExternalInput")
    mt_d = nc.dram_tensor("mt", [128, HT * B_LOC], mybir.dt.uint8,
                          kind="ExternalInput")
    bt_d = nc.dram_tensor("bt", [128, HT * B_LOC], mybir.dt.uint8,
                          kind="ExternalInput")
    # weights pre-swizzled: [p, ht, kt, c] -> [128, HT*KT*128]
    w1_d = nc.dram_tensor("w1", [128, HT * KT1 * 128], f32r, kind="ExternalInput")
    w2x_d = nc.dram_tensor("w2x", [128, HT * KT1 * 128], bf16,
                           kind="ExternalInput")
    w2z_d = nc.dram_tensor("w2z", [128, HT * KT1 * 128], fp8,
                           kind="ExternalInput")

    zo_d = nc.dram_tensor("zo", [128, HT * B_LOC], mybir.dt.uint8,
                          kind="ExternalOutput")
    vo_d = nc.dram_tensor("vo", [128, HT * B_LOC], bf16, kind="ExternalOutput")
    io_d = nc.dram_tensor("io", [128, HT * B_LOC], bf16, kind="ExternalOutput")
    ro_d = nc.dram_tensor("ro", [128, HT * B_LOC], bf16, kind="ExternalOutput")

    with tile.TileContext(nc) as tc:
        with (
            tc.tile_pool(name="resid", bufs=1) as resid,
            tc.tile_pool(name="w1pool", bufs=5) as w1pool,
            tc.tile_pool(name="w2xpool", bufs=5) as w2xpool,
            tc.tile_pool(name="w2zpool", bufs=4) as w2zpool,
            tc.tile_pool(name="spool", bufs=3) as spool,
            tc.tile_pool(name="epool", bufs=2) as epool,
            tc.tile_pool(name="opool", bufs=2) as opool,
            tc.tile_pool(name="pspool", bufs=2, space="PSUM") as pspool,
        ):
            u8 = mybir.dt.uint8
            vt_sb = resid.tile([128, KT1 * B_LOC], f32r)
            zt_sb = resid.tile([128, KT1, B_LOC], fp8)
            xt_sb = resid.tile([128, KT1 * B_LOC], bf16)
            p2x_sb = resid.tile([128, HT * B_LOC], bf16)
            fives = resid.tile([128, 2 * B_LOC], bf16)
            nc.vector.memset(fives[:], 5.0)
            zeros = resid.tile([128, 2 * B_LOC], u8)
            nc.vector.memset(zeros[:], 0)

            # ---- helpers ----
            KB = B_LOC                # one k-tile worth of rhs columns
            w1_t = {}
            w2x_t = {}
            w2z_t = {}
            i2_t = {}
            r2_t = {}
            ps1_t = {}
            ps2_t = {}

            def load_w1(h):
                t = w1pool.tile([128, KT1 * 128], f32r, name="w1_sb")
                nc.sync.dma_start(
                    t[:], w1_d[:, h * KT1 * 128:(h + 1) * KT1 * 128])
                w1_t[h] = t

            def load_w2(h, half):
                off = h * KT1 * 128
                if half == 0:
                    t = w2xpool.tile([128, KT1 * 128], bf16, name="w2_sb")
                    nc.sync.dma_start(
                        t[:], w2x_d[:, off:off + KT1 * 128])
                    w2x_t[h] = t
                else:
                    t = w2zpool.tile([128, KT1, 128], fp8, name="w2z_sb")
                    nc.sync.dma_start(
                        t[:, :, :], w2z_d[:, off:off + KT1 * 128])
                    w2z_t[h] = t

            def load_states(hp):
                pw = slice(2 * hp * B_LOC, (2 * hp + 2) * B_LOC)
                i2 = spool.tile([128, 2 * B_LOC], bf16, name="i2")
                nc.sync.dma_start(i2[:], it_d[:, pw])
                i2_t[hp] = i2
                m2i = spool.tile([128, 2 * B_LOC], u8, name="m2i")
                nc.sync.dma_start(m2i[:], mt_d[:, pw])
                b2u = spool.tile([128, 2 * B_LOC], u8, name="b2u")
                nc.sync.dma_start(b2u[:], bt_d[:, pw])
                r2_t[hp] = (m2i, b2u)

            def derive_states(hp):
                """Invert the refrac mask and dequantize the rho base on
                DVE (cheap ops traded for 2 MiB of HBM traffic). Called
                at pair-body start, well after the DMAs landed, so the
                in-order DVE queue never stalls on them."""
                m2i, b2u = r2_t[hp]
                n2i = spool.tile([128, 2 * B_LOC], u8, name="n2i")
                nc.scalar.activation(n2i[:], m2i[:], F.Copy,
                                     bias=1.0, scale=-1.0)
                rob = spool.tile([128, 2 * B_LOC], bf16, name="rob")
                nc.scalar.activation(rob[:], b2u[:], F.Copy,
                                     bias=0.0, scale=BT_SCALE)
                r2_t[hp] = (m2i, n2i, rob)

            def mm1(h):
                """ps1[h] = v @ g_coupling.T accumulation, all k-tiles."""
                ps1 = pspool.tile([128, B_LOC], f32, name="ps1", bufs=4)
                ps1_t[h] = ps1
                w1_sb = w1_t.pop(h)
                for k in range(KT1):
                    nc.tensor.matmul(
                        ps1[:], w1_sb[:, k * 128:(k + 1) * 128],
                        vt_sb[:, k * KB:(k + 1) * KB],
                        start=(k == 0), stop=(k == KT1 - 1))

            def mm2(h, half, k0, k1):
                """x-part (half=0) or z-part (half=1) of the i_new GEMM.
                The z-part runs in fp8 e4m3 with DoubleRow (2 weights per
                PE cell, two k-tiles per instruction at 0.5 cyc/row)."""
                name = "ps2x" if half == 0 else "ps2z"
                if (h, half) not in ps2_t:
                    ps2_t[(h, half)] = pspool.tile(
                        [128, B_LOC], f32, name=name)
                pso = ps2_t[(h, half)]
                if half == 0:
                    w2_sb = w2x_t[h]
                    for k in range(k0, k1):
                        nc.tensor.matmul(
                            pso[:], w2_sb[:, k * 128:(k + 1) * 128],
                            xt_sb[:, k * KB:(k + 1) * KB],
                            start=(k == 0), stop=(k == KT1 - 1))
                else:
                    w2_sb = w2z_t[h]
                    for j in range(k0 // 2, k1 // 2):
                        nc.tensor.matmul(
                            pso[:], w2_sb[:, 2 * j:2 * j + 2, :],
                            zt_sb[:, 2 * j:2 * j + 2, :],
                            start=(j == 0), stop=(j == KT1 // 2 - 1),
                            perf_mode=mybir.MatmulPerfMode.DoubleRow)

            def evac_x(h):
                """Park the x-part partial sum in SBUF (bf16), on ACT."""
                hw = slice(h * B_LOC, (h + 1) * B_LOC)
                nc.scalar.activation(p2x_sb[:, hw], ps2_t.pop((h, 0))[:],
                                     F.Copy, bias=0.0, scale=1.0)
                w2x_t.pop(h)

            # ---- phase A: x-half of the i_new GEMM (bf16, DMA-dense;
            # partial sums parked in SBUF so only 2 PSUM banks are used) ----
            load_w2(0, 0)
            for c in range(2):
                nc.sync.dma_start(xt_sb[:, 2 * c * KB:2 * (c + 1) * KB],
                                  xt_d[:, 2 * c * KB:2 * (c + 1) * KB])
                mm2(0, 0, 2 * c, 2 * (c + 1))
            load_w2(1, 0)
            for c in range(2, 8):
                nc.sync.dma_start(xt_sb[:, 2 * c * KB:2 * (c + 1) * KB],
                                  xt_d[:, 2 * c * KB:2 * (c + 1) * KB])
                mm2(0, 0, 2 * c, 2 * (c + 1))
            mm2(1, 0, 0, KT1)
            evac_x(0)
            for h in range(2, HT):
                load_w2(h, 0)
                mm2(h, 0, 0, KT1)
                evac_x(h - 1)
            evac_x(HT - 1)

            # ---- phase B: z-half + coupling GEMMs + LIF elementwise ----
            load_w1(0)
            load_w2(0, 1)
            nc.sync.dma_start(zt_sb[:, :8, :], zt_d[:, :8 * KB])
            nc.sync.dma_start(zt_sb[:, 8:, :], zt_d[:, 8 * KB:])
            load_w2(1, 1)
            load_states(0)
            for c in range(4):
                nc.sync.dma_start(vt_sb[:, 4 * c * KB:4 * (c + 1) * KB],
                                  vt_d[:, 4 * c * KB:4 * (c + 1) * KB])
            load_w1(1)
            load_states(1)

            u2_t = {}

            def act_u2(hp):
                """u2 = 0.1*i on ACT, prefetched a pair ahead."""
                u2 = epool.tile([128, 2 * B_LOC], f32, name="u2")
                pw = slice(2 * hp * B_LOC, (2 * hp + 2) * B_LOC)
                nc.scalar.activation(u2[:], i2_t[hp][:], F.Copy,
                                     bias=0.0, scale=0.1)
                u2_t[hp] = u2

            act_u2(0)

            def elem_io(hp, hh0, hh1, i2):
                """i_new = 0.8*i + ps2z + parked x-part; frees ps2z banks."""
                n = (hh1 - hh0) * B_LOC
                pw = slice((2 * hp + hh0) * B_LOC,
                           (2 * hp + hh1) * B_LOC)
                iob = opool.tile([128, n], bf16, name="iob")
                for hh in range(hh0, hh1):
                    h = 2 * hp + hh
                    dst = slice((hh - hh0) * B_LOC, (hh - hh0 + 1) * B_LOC)
                    src = slice(hh * B_LOC, (hh + 1) * B_LOC)
                    t = epool.tile([128, B_LOC], f32, name="zt_des")
                    nc.vector.scalar_tensor_tensor(
                        t[:], in0=ps2_t.pop((h, 1))[:], scalar=1.0 / 1024.0,
                        in1=p2x_sb[:, h * B_LOC:(h + 1) * B_LOC],
                        op0=A.mult, op1=A.add)
                    nc.vector.scalar_tensor_tensor(
                        iob[:, dst], in0=i2[:, src], scalar=0.8,
                        in1=t[:], op0=A.mult, op1=A.add)
                nc.scalar.dma_start(io_d[:, pw], iob[:])

            def elem(hp, hh0, hh1, i2, r2, u2, skip_io=False):
                """LIF/refrac elementwise for h-range [2hp+hh0, 2hp+hh1)."""
                n = (hh1 - hh0) * B_LOC
                sw = slice(hh0 * B_LOC, hh1 * B_LOC)            # pair-local
                pw = slice((2 * hp + hh0) * B_LOC,
                           (2 * hp + hh1) * B_LOC)              # global
                v2 = vt_sb[:, pw].bitcast(f32)
                m2i_p, n2i_p, rob_p = r2
                m2i = m2i_p[:, sw]
                n2i = n2i_p[:, sw]
                rob = rob_p[:, sw]

                if not skip_io:
                    elem_io(hp, hh0, hh1, i2)

                # vdec = 0.1*i + ps1   (ps1 already carries 0.9*v via
                # the identity folded into the coupling weights)
                vdec = epool.tile([128, n], f32, name="vdec")
                for hh in range(hh0, hh1):
                    hw = slice((hh - hh0) * B_LOC, (hh - hh0 + 1) * B_LOC)
                    uw = slice(hh * B_LOC, (hh + 1) * B_LOC)
                    nc.vector.tensor_add(vdec[:, hw], u2[:, uw],
                                         ps1_t.pop(2 * hp + hh)[:])

                # spikes: z = (vdec > 1)*not_refrac in one fused stt,
                # using the host-shipped inverted mask
                z2i = epool.tile([128, n], u8, name="z2i")
                nc.vector.scalar_tensor_tensor(
                    z2i[:], in0=vdec[:], scalar=1.0, in1=n2i,
                    op0=A.is_gt, op1=A.mult)

                # v_new = (vdec<=1)*vdec, held v where refractory; the
                # predicated copy needs matching f32 dtypes, so narrow to
                # the bf16 output tile on the (idle) ACT engine afterwards
                vof = epool.tile([128, n], f32, name="vof")
                nc.vector.scalar_tensor_tensor(
                    vof[:], in0=vdec[:], scalar=1.0, in1=vdec[:],
                    op0=A.is_le, op1=A.mult)
                nc.vector.copy_predicated(vof[:], m2i, v2)
                vob = opool.tile([128, n], bf16, name="vob")
                nc.scalar.activation(vob[:], vof[:], F.Copy,
                                     bias=0.0, scale=1.0)

                # rho_new = relu(rho - 1) (host-precomputed), 5 where spiking
                nc.vector.copy_predicated(rob, z2i[:], fives[:, :n])

                if hp == HP - 1:
                    # final outputs over both HWDGE queues for a short tail
                    nc.scalar.dma_start(zo_d[:, pw], z2i[:])
                    nc.sync.dma_start(vo_d[:, pw], vob[:])
                    nc.scalar.dma_start(ro_d[:, pw], rob)
                else:
                    nc.scalar.dma_start(zo_d[:, pw], z2i[:])
                    nc.scalar.dma_start(vo_d[:, pw], vob[:])
                    nc.scalar.dma_start(ro_d[:, pw], rob)

            for hp in range(HP):
                i2 = i2_t.pop(hp)
                derive_states(hp)
                r2 = r2_t.pop(hp)
                u2 = u2_t.pop(hp)
                last = hp == HP - 1

                # prefetch next pair's weights/states
                if hp + 1 < HP:
                    load_w1(2 * hp + 2)
                    load_w2(2 * hp + 2, 1)
                mm2(2 * hp, 1, 0, KT1)
                mm1(2 * hp)
                if hp + 1 < HP:
                    load_w1(2 * hp + 3)
                    load_w2(2 * hp + 3, 1)
                if hp + 2 < HP:
                    load_states(hp + 2)
                if hp + 1 < HP:
                    act_u2(hp + 1)
                if last:
                    # split the final pair per-h so h14's elementwise
                    # overlaps h15's GEMM, and make the h15 z-GEMM the
                    # very last PE work — its only consumer is the cheap
                    # io evacuation, so the kernel tail is short
                    elem(hp, 0, 1, i2, r2, u2)
                    mm1(2 * hp + 1)
                    elem(hp, 1, 2, i2, r2, u2, skip_io=True)
                    mm2(2 * hp + 1, 1, 0, KT1)
                    elem_io(hp, 1, 2, i2)
                else:
                    mm2(2 * hp + 1, 1, 0, KT1)
                    mm1(2 * hp + 1)
                    elem(hp, 0, 2, i2, r2, u2)

    nc.compile()
    return nc


def _sw_act(x, dtype=F32):
    """[B_LOC, K] -> [128, KT*B_LOC] with layout [p, kt, b]."""
    a = np.ascontiguousarray(x.T).reshape(KT1, 128, B_LOC).transpose(1, 0, 2)
    return np.ascontiguousarray(a).astype(dtype).reshape(128, KT1 * B_LOC)


def _unsw(y):
    """[128, HT*B_LOC] ([p, ht, b]) -> [B_LOC, H] float32."""
    a = y.astype(np.float32).reshape(128, HT, B_LOC)
    a = a.transpose(1, 0, 2).reshape(H, B_LOC)
    return a.T


def _sw_w(WT, kt, dtype=F32):
    """WT=[K,H] -> [128, HT*kt*128] with layout [p, ht, kt, c]."""
    a = WT.reshape(kt, 128, HT, 128)              # [k, p, h, c]
    return np.ascontiguousarray(
        a.transpose(1, 2, 0, 3)).astype(dtype).reshape(128, HT * kt * 128)


def swizzle_weights(input_weights, recurrent_weights, g_coupling):
    # fold the 0.9*v membrane-leak term into the coupling GEMM:
    # ps1 = v @ (G + 0.9 I).T = v @ G.T + 0.9 v (saves a DVE op per pair;
    # the f32r rounding on the diagonal adds ~2e-4*|v|, a few flips)
    G = np.ascontiguousarray(
        np.asarray(g_coupling, np.float32).T
        + 0.9 * np.eye(H, dtype=np.float32))
    Wx = np.ascontiguousarray(np.asarray(input_weights, np.float32).T)
    Wz = np.ascontiguousarray(
        np.asarray(recurrent_weights, np.float32).T * W_SCALE)
    return _sw_w(G, KT1), _sw_w(Wx, KT1, BF16), _sw_w(Wz, KT1, FP8)


def make_core_inputs(inp, z, v, i, rho, Wi=None, Wr=None, G=None, w=None):
    """Per-core in_map from this core's [B_LOC, *] shards + full weights."""
    if w is None:
        w = swizzle_weights(Wi, Wr, G)
    w1, w2 = w[0], (w[1], w[2])
    return {
        "vt": _sw_act(v), "zt": _sw_act(z * Z_SCALE, FP8),
        "xt": _sw_act(inp, BF16),
        "it": _sw_act(i, BF16),
        "mt": _sw_act((rho > 0), np.uint8),
        "bt": _sw_act(np.clip(np.round(
            np.maximum(rho - 1.0, 0.0) * BT_QSCALE), 0, 255), np.uint8),
        "w1": w1, "w2x": w2[0], "w2z": w2[1],
    }


def unpack_core_outputs(outs):
    """Dict of raw per-core outputs -> (z_new, v_new, i_new, rho_new)."""
    return tuple(_unsw(outs[name]) for name in OUT_NAMES)


def kernel(inp, z, v, i, rho, input_weights, recurrent_weights, g_coupling):
    inp = np.ascontiguousarray(inp, dtype=np.float32)
    z = np.ascontiguousarray(z, dtype=np.float32)
    v = np.ascontiguousarray(v, dtype=np.float32)
    i = np.ascontiguousarray(i, dtype=np.float32)
    rho = np.ascontiguousarray(rho, dtype=np.float32)

    if "nc" not in _cache:
        _cache["nc"] = build()
    nc = _cache["nc"]
    wkey = (id(input_weights), id(recurrent_weights), id(g_coupling))
    if _cache.get("wkey") != wkey:
        _cache["w"] = swizzle_weights(
            input_weights, recurrent_weights, g_coupling)
        _cache["wkey"] = wkey

    def sw_all(x, dtype=F32):
        """[B, K] -> per-core [128, KT*B_LOC] swizzles in one pass."""
        a = np.ascontiguousarray(x.T).reshape(KT1, 128, NCORES, B_LOC)
        a = np.ascontiguousarray(a.transpose(2, 1, 0, 3)).astype(dtype)
        return a.reshape(NCORES, 128, KT1 * B_LOC)

    sv, sz, sx = sw_all(v), sw_all(z * Z_SCALE, FP8), sw_all(inp, BF16)
    si = sw_all(i, BF16)
    sm = sw_all((rho > 0), np.uint8)
    sb = sw_all(np.clip(np.round(
        np.maximum(rho - 1.0, 0.0) * BT_QSCALE), 0, 255), np.uint8)
    w1, w2x, w2z = _cache["w"]
    in_maps = [{"vt": sv[c], "zt": sz[c], "xt": sx[c],
                "it": si[c], "mt": sm[c], "bt": sb[c],
                "w1": w1, "w2x": w2x, "w2z": w2z}
               for c in range(NCORES)]

    res = bass_utils.run_bass_kernel_spmd(
        nc, in_maps, core_ids=list(range(NCORES)),
        trace=bool(int(os.environ.get("LIF_TRACE", "0"))),
    )
    _cache["last_results"] = res

    outs = []
    for name in OUT_NAMES:
        full = np.empty((B, H), np.float32)
        for c in range(NCORES):
            full[c * B_LOC:(c + 1) * B_LOC] = _unsw(res.results[c][name])
        outs.append(full)
    return np.stack(outs)
